# revision 9
# baseline (speedup 1.0000x reference)
"""Trainium2 Bass kernel for the NeuralCamera sampling module.

Pipeline per ray (fully vectorized, gather-free):
  1. coarse integrate -> per-ray cdf (63 knots) via exp/scan ops
  2. per-bin tables a_k, m_k, b_k encoding the piecewise-linear inverse cdf,
     with degenerate bins (denom < 1e-5) handled exactly via a step form:
        G_k(u) = min(relu((u - a_k) + b_k) * m_k, 1)
        normal: a=c_k,     m=1/denom, b=0
        degen:  a=c_{k+1}, m=2^100,   b=2^-100   (so G = [u >= c_{k+1}] exactly)
     z(u) = bins_0 + h * sum_k G_k(u)   (uniform z grid -> bins arithmetic)
  3. bitonic sort of u (map is monotone -> sorted z_fine directly)
  4. fine integrate -> rgb / acc / depth

Data parallel over rays: 262144 rays sharded across 8 NeuronCores.
Layout: ray r -> partition r%128, free slot r//128 (within each macro-tile).
"""

import numpy as np

R_FULL = 262144
S = 64
NCORES = 8
R_CORE = R_FULL // NCORES      # 32768
T = 16                          # ray-groups per partition per macro tile
MACRO = 128 * T                 # 2048 rays per macro tile
NMACRO = R_CORE // MACRO        # 16

_CACHE = {}


def _build_program(r_core=R_CORE, n_devices=NCORES, debug=False):
    import concourse.bacc as bacc
    import concourse.tile as tile
    import concourse.mybir as mybir
    from concourse.mybir import AluOpType as alu
    from concourse.mybir import ActivationFunctionType as act

    nmacro = r_core // MACRO
    assert nmacro * MACRO == r_core

    f32 = mybir.dt.float32
    nc = bacc.Bacc("TRN2", target_bir_lowering=False, debug=debug,
                   num_devices=n_devices)

    d_dc = nc.dram_tensor("density_coarse", [r_core, S], f32, kind="ExternalInput")
    d_z = nc.dram_tensor("z_vals", [r_core, S], f32, kind="ExternalInput")
    d_rd = nc.dram_tensor("rays_d", [r_core, 3], f32, kind="ExternalInput")
    d_u = nc.dram_tensor("u", [r_core, S], f32, kind="ExternalInput")
    d_df = nc.dram_tensor("density_fine", [r_core, S], f32, kind="ExternalInput")
    d_cf = nc.dram_tensor("color_fine", [r_core, S, 3], f32, kind="ExternalInput")
    d_rgb = nc.dram_tensor("rgb", [r_core, 3], f32, kind="ExternalOutput")
    d_acc = nc.dram_tensor("acc", [r_core], f32, kind="ExternalOutput")
    d_dep = nc.dram_tensor("depth", [r_core], f32, kind="ExternalOutput")
    d_zf = nc.dram_tensor("z_fine", [r_core, S], f32, kind="ExternalOutput")

    P = 128

    def dview(handle, m, pat, **kw):
        a = handle.ap()[m * MACRO:(m + 1) * MACRO]
        return a.rearrange(pat, p=P, **kw)

    with tile.TileContext(nc) as tc:
        with tc.tile_pool(name="const", bufs=1) as cpool, \
             tc.tile_pool(name="io", bufs=2) as io, \
             tc.tile_pool(name="work", bufs=1) as wk:

            # constant scan reset-pattern tiles (built once)
            p1 = cpool.tile([P, T, 64], f32)      # cumprod pattern: col0=1 else -1
            nc.gpsimd.memset(p1[:, :, 0:1], 1.0)
            nc.gpsimd.memset(p1[:, :, 1:64], -1.0)
            p2 = cpool.tile([P, T, 63], f32)      # cumsum pattern: col62=0 else 1
            nc.gpsimd.memset(p2[:, :, 0:62], 1.0)
            nc.gpsimd.memset(p2[:, :, 62:63], 0.0)
            pf = cpool.tile([P, T, 66], f32)      # fine cumprod pattern
            nc.gpsimd.memset(pf[:, :, 0:1], 1.0)
            nc.gpsimd.memset(pf[:, :, 1:66], -1.0)

            for m in range(nmacro):
                # ---------------- DMA in ----------------
                tz = io.tile([P, T, 64], f32, tag="tz")
                nc.sync.dma_start(tz[:], dview(d_z, m, "(f p) s -> p f s"))
                tdc = io.tile([P, T, 64], f32, tag="tdc")
                nc.sync.dma_start(tdc[:], dview(d_dc, m, "(f p) s -> p f s"))
                trd = io.tile([P, T, 3], f32, tag="trd")
                nc.sync.dma_start(trd[:], dview(d_rd, m, "(f p) c -> p f c"))
                tu = io.tile([P, T, 64], f32, tag="tu")
                nc.sync.dma_start(tu[:], dview(d_u, m, "(f p) s -> p f s"))
                tdf = io.tile([P, T, 65], f32, tag="tdf")
                nc.sync.dma_start(tdf[:, :, 0:64], dview(d_df, m, "(f p) s -> p f s"))
                tcf = io.tile([P, T, 65, 3], f32, tag="tcf")
                nc.sync.dma_start(tcf[:, :, 0:64, :],
                                  dview(d_cf, m, "(f p) s c -> p f s c"))

                # ---------------- phase A: coarse -> cdf & tables ----------
                sq = wk.tile([P, T, 3], f32, tag="sq")
                nc.scalar.activation(sq[:], trd[:], act.Square)
                nsum = wk.tile([P, T, 1], f32, tag="nsum")
                nc.vector.reduce_sum(nsum[:, :, 0], sq[:], axis=mybir.AxisListType.X)
                nrm = wk.tile([P, T, 1], f32, tag="nrm")   # |rays_d|
                nc.scalar.activation(nrm[:], nsum[:], act.Sqrt)

                b0 = wk.tile([P, T, 1], f32, tag="b0")      # first bin center
                nc.vector.tensor_tensor(b0[:], tz[:, :, 0:1], tz[:, :, 1:2], op=alu.add)
                nc.vector.tensor_scalar_mul(b0[:], b0[:], 0.5)
                hh = wk.tile([P, T, 1], f32, tag="hh")      # bin width
                nc.vector.tensor_tensor(hh[:], tz[:, :, 1:2], tz[:, :, 0:1],
                                        op=alu.subtract)

                s1 = wk.tile([P, T, 63], f32, tag="s1")
                nc.vector.tensor_tensor(s1[:], tz[:, :, 1:64], tz[:, :, 0:63],
                                        op=alu.subtract)           # zdiff
                nc.vector.tensor_scalar_mul(s1[:], s1[:], 100.0)
                nc.vector.tensor_tensor(s1[:], s1[:],
                                        nrm[:].broadcast_to([P, T, 63]),
                                        op=alu.mult)               # dists
                nc.vector.tensor_tensor(s1[:], tdc[:, :, 0:63], s1[:],
                                        op=alu.mult)               # sigma*dist
                s2 = wk.tile([P, T, 63], f32, tag="s2")
                nc.scalar.activation(s2[:], s1[:], act.Relu)
                e1 = wk.tile([P, T, 63], f32, tag="e1")
                nc.scalar.activation(e1[:], s2[:], act.Exp, scale=-1.0)
                alpha = wk.tile([P, T, 63], f32, tag="alpha")
                nc.vector.tensor_scalar(alpha[:], e1[:], -1.0, 1.0,
                                        op0=alu.mult, op1=alu.add)  # 1-e

                mbuf = wk.tile([P, T, 64], f32, tag="mbuf")
                nc.gpsimd.memset(mbuf[:, :, 0:1], 1.0)
                nc.gpsimd.memset(mbuf[:, :, 63:64], 0.0)
                # m = (1 - alpha) + 1e-10, matching reference rounding
                nc.vector.tensor_scalar(s2[:, :, 0:62], alpha[:, :, 0:62],
                                        -1.0, 1.0, op0=alu.mult, op1=alu.add)
                nc.vector.tensor_scalar(mbuf[:, :, 1:63], s2[:, :, 0:62],
                                        1.0e-10, None, op0=alu.add)
                trans = wk.tile([P, T, 64], f32, tag="trans")
                nc.vector.tensor_tensor_scan(
                    trans.rearrange("p a b -> p (a b)"),
                    mbuf.rearrange("p a b -> p (a b)"),
                    p1.rearrange("p a b -> p (a b)"),
                    1.0, op0=alu.mult, op1=alu.max)

                wbuf = wk.tile([P, T, 63], f32, tag="wbuf")
                nc.gpsimd.memset(wbuf[:, :, 62:63], 0.0)
                nc.vector.tensor_tensor(wbuf[:, :, 0:62], alpha[:, :, 1:63],
                                        trans[:, :, 1:63], op=alu.mult)
                nc.vector.tensor_scalar(wbuf[:, :, 0:62], wbuf[:, :, 0:62],
                                        1.0e-5, None, op0=alu.add)
                Tsum = wk.tile([P, T, 1], f32, tag="Tsum")
                nc.vector.reduce_sum(Tsum[:, :, 0], wbuf[:, :, 0:62],
                                     axis=mybir.AxisListType.X)
                rT = wk.tile([P, T, 1], f32, tag="rT")
                nc.vector.reciprocal(rT[:], Tsum[:])
                nc.vector.tensor_tensor(wbuf[:, :, 0:62], wbuf[:, :, 0:62],
                                        rT[:].broadcast_to([P, T, 62]),
                                        op=alu.mult)               # pdf
                ws = wk.tile([P, T, 63], f32, tag="ws")
                nc.vector.tensor_tensor_scan(
                    ws.rearrange("p a b -> p (a b)"),
                    wbuf.rearrange("p a b -> p (a b)"),
                    p2.rearrange("p a b -> p (a b)"),
                    0.0, op0=alu.add, op1=alu.mult)

                cdf = wk.tile([P, T, 63], f32, tag="cdf")
                nc.gpsimd.memset(cdf[:, :, 0:1], 0.0)
                nc.vector.tensor_copy(cdf[:, :, 1:63], ws[:, :, 0:62])

                den = wk.tile([P, T, 62], f32, tag="den")
                nc.vector.tensor_tensor(den[:], cdf[:, :, 1:63], cdf[:, :, 0:62],
                                        op=alu.subtract)
                dg = wk.tile([P, T, 62], f32, tag="dg")
                nc.vector.tensor_scalar(dg[:], den[:], 1.0e-5, None, op0=alu.is_lt)
                At = wk.tile([P, T, 62], f32, tag="At")
                nc.vector.tensor_tensor(At[:], dg[:], den[:], op=alu.mult)
                nc.vector.tensor_tensor(At[:], At[:], cdf[:, :, 0:62], op=alu.add)
                # degen entries -> nextdown(c_{k+1}) via *(1 - 2^-24)
                fdg = wk.tile([P, T, 62], f32, tag="fdg")
                nc.vector.tensor_scalar(fdg[:], dg[:], float(-(2.0 ** -24)), 1.0,
                                        op0=alu.mult, op1=alu.add)
                nc.vector.tensor_tensor(At[:], At[:], fdg[:], op=alu.mult)
                Mt = wk.tile([P, T, 62], f32, tag="Mt")
                nc.vector.reciprocal(Mt[:], den[:])
                nc.vector.scalar_tensor_tensor(Mt[:], dg[:], float(2.0 ** 100), Mt[:],
                                               op0=alu.mult, op1=alu.add)

                # ---------------- phase B: bitonic sort of u ----------------
                vb = wk.tile([P, T, 64], f32, tag="vb")
                bufs = [tu, vb]
                si = 0
                for ph in range(1, 7):
                    for d in [1 << e for e in range(ph - 1, -1, -1)]:
                        src, dst = bufs[si % 2], bufs[(si + 1) % 2]
                        si += 1
                        sf = src.rearrange("p a b -> p (a b)")
                        df_ = dst.rearrange("p a b -> p (a b)")
                        if ph < 6:
                            W = 1 << (ph + 1)
                            H = 1 << ph
                            sv = sf.rearrange("p (a w) -> p a w", w=W)
                            dv = df_.rearrange("p (a w) -> p a w", w=W)
                            for off, lo_is_min in ((0, True), (H, False)):
                                region_s = sv[:, :, off:off + H].rearrange(
                                    "p a (g t) -> p a g t", t=2 * d)
                                region_d = dv[:, :, off:off + H].rearrange(
                                    "p a (g t) -> p a g t", t=2 * d)
                                lo_s = region_s[:, :, :, 0:d]
                                hi_s = region_s[:, :, :, d:2 * d]
                                lo_d = region_d[:, :, :, 0:d]
                                hi_d = region_d[:, :, :, d:2 * d]
                                nc.vector.tensor_tensor(
                                    lo_d, lo_s, hi_s,
                                    op=alu.min if lo_is_min else alu.max)
                                nc.vector.tensor_tensor(
                                    hi_d, lo_s, hi_s,
                                    op=alu.max if lo_is_min else alu.min)
                        else:
                            rs = sf.rearrange("p (g t) -> p g t", t=2 * d)
                            rd_ = df_.rearrange("p (g t) -> p g t", t=2 * d)
                            nc.vector.tensor_tensor(rd_[:, :, 0:d], rs[:, :, 0:d],
                                                    rs[:, :, d:2 * d], op=alu.min)
                            nc.vector.tensor_tensor(rd_[:, :, d:2 * d], rs[:, :, 0:d],
                                                    rs[:, :, d:2 * d], op=alu.max)
                v = bufs[si % 2]   # sorted u (21 stages -> odd -> vb)

                # ---------------- phase B2: telescoping inverse-cdf ----------
                # accG = sum_k min(relu(v - A'_k) * M_k, 1);  z = b0 + h*accG
                zpad = wk.tile([P, T, 65], f32, tag="zpad")
                accg = wk.tile([P, T, 64], f32, tag="accg")
                accs = wk.tile([P, T, 64], f32, tag="accs")
                nc.vector.memset(accg[:], 0.0)
                ntb = 4          # rotate sub/scratch buffers for overlap
                tbs = [wk.tile([P, T, 64], f32, tag=f"tb{i}", name=f"tb{i}")
                       for i in range(ntb)]
                sbs = [wk.tile([P, T, 64], f32, tag=f"sb{i}", name=f"sb{i}")
                       for i in range(ntb)]
                accp = [accg[:], accs[:]]
                for k in range(62):
                    a_b = At[:, :, k:k + 1].broadcast_to([P, T, 64])
                    m_b = Mt[:, :, k:k + 1].broadcast_to([P, T, 64])
                    tb = tbs[k % ntb]
                    sb = sbs[k % ntb]
                    nc.gpsimd.tensor_tensor(tb[:], v[:], a_b, op=alu.subtract)
                    nc.vector.scalar_tensor_tensor(sb[:], tb[:], 0.0, m_b,
                                                   op0=alu.max, op1=alu.mult)
                    nc.vector.scalar_tensor_tensor(accp[(k + 1) % 2], sb[:], 1.0,
                                                   accp[k % 2],
                                                   op0=alu.min, op1=alu.add)
                # 62 steps -> final accG in accp[0] == accg
                nc.vector.tensor_tensor(accg[:], accg[:],
                                        hh[:].broadcast_to([P, T, 64]),
                                        op=alu.mult)
                nc.vector.tensor_tensor(zpad[:, :, 0:64], accg[:],
                                        b0[:].broadcast_to([P, T, 64]),
                                        op=alu.add)

                # ---------------- phase C: fine integrate ----------------
                nc.gpsimd.memset(zpad[:, :, 64:65], 1.0e8)
                nc.gpsimd.memset(tdf[:, :, 64:65], 1.0e8)
                nc.gpsimd.memset(tcf[:, :, 64:65, :], 1.0)

                distf = wk.tile([P, T, 65], f32, tag="distf")
                nc.vector.tensor_tensor(distf[:, :, 0:64], zpad[:, :, 1:65],
                                        zpad[:, :, 0:64], op=alu.subtract)
                nc.vector.tensor_scalar_mul(distf[:, :, 0:64],
                                            distf[:, :, 0:64], 100.0)
                nc.vector.tensor_tensor(distf[:, :, 0:64], distf[:, :, 0:64],
                                        nrm[:].broadcast_to([P, T, 64]),
                                        op=alu.mult)
                nc.vector.tensor_scalar_mul(distf[:, :, 64:65], nrm[:], 1.0e10)
                qf = wk.tile([P, T, 65], f32, tag="qf")
                nc.vector.tensor_tensor(qf[:], tdf[:], distf[:], op=alu.mult)
                nc.scalar.activation(qf[:], qf[:], act.Relu)
                ef = wk.tile([P, T, 65], f32, tag="ef")
                nc.scalar.activation(ef[:], qf[:], act.Exp, scale=-1.0)
                alphaf = wk.tile([P, T, 65], f32, tag="alphaf")
                nc.vector.tensor_scalar(alphaf[:], ef[:], -1.0, 1.0,
                                        op0=alu.mult, op1=alu.add)
                mfb = wk.tile([P, T, 66], f32, tag="mfb")
                nc.gpsimd.memset(mfb[:, :, 0:1], 1.0)
                nc.gpsimd.memset(mfb[:, :, 65:66], 0.0)
                # mf = (1 - alphaf) + 1e-10
                nc.vector.tensor_scalar(qf[:, :, 0:64], alphaf[:, :, 0:64],
                                        -1.0, 1.0, op0=alu.mult, op1=alu.add)
                nc.vector.tensor_scalar(mfb[:, :, 1:65], qf[:, :, 0:64],
                                        1.0e-10, None, op0=alu.add)
                trf = wk.tile([P, T, 66], f32, tag="trf")
                nc.vector.tensor_tensor_scan(
                    trf.rearrange("p a b -> p (a b)"),
                    mfb.rearrange("p a b -> p (a b)"),
                    pf.rearrange("p a b -> p (a b)"),
                    1.0, op0=alu.mult, op1=alu.max)
                wf = wk.tile([P, T, 65], f32, tag="wf")
                nc.vector.tensor_tensor(wf[:], alphaf[:], trf[:, :, 0:65],
                                        op=alu.mult)

                acct = wk.tile([P, T, 1], f32, tag="acct")
                nc.vector.reduce_sum(acct[:, :, 0], wf[:], axis=mybir.AxisListType.X)
                wz = wk.tile([P, T, 65], f32, tag="wz")
                nc.vector.tensor_tensor(wz[:], wf[:], zpad[:], op=alu.mult)
                dept = wk.tile([P, T, 1], f32, tag="dept")
                nc.vector.reduce_sum(dept[:, :, 0], wz[:], axis=mybir.AxisListType.X)
                nc.vector.tensor_tensor(tcf[:], wf[:].unsqueeze(3).broadcast_to(
                    [P, T, 65, 3]), tcf[:], op=alu.mult)
                rgbt = wk.tile([P, T, 3], f32, tag="rgbt")
                nc.vector.reduce_sum(rgbt[:],
                                     tcf.rearrange("p a s c -> p a c s"),
                                     axis=mybir.AxisListType.X)

                # ---------------- DMA out ----------------
                nc.sync.dma_start(dview(d_zf, m, "(f p) s -> p f s"),
                                  zpad[:, :, 0:64])
                nc.sync.dma_start(dview(d_rgb, m, "(f p) c -> p f c"), rgbt[:])
                nc.sync.dma_start(dview(d_acc, m, "(f p) -> p f"), acct[:, :, 0])
                nc.sync.dma_start(dview(d_dep, m, "(f p) -> p f"), dept[:, :, 0])

    nc.compile()
    return nc


def _get_program():
    if "nc" not in _CACHE:
        _CACHE["nc"] = _build_program()
    return _CACHE["nc"]


def kernel(density_coarse, z_vals, rays_d, u, density_fine, color_fine):
    from concourse.bass_utils import run_bass_kernel_spmd

    nc = _get_program()
    dc = np.ascontiguousarray(
        np.asarray(density_coarse, np.float32).reshape(R_FULL, S))
    zv = np.ascontiguousarray(np.asarray(z_vals, np.float32))
    rd = np.ascontiguousarray(np.asarray(rays_d, np.float32))
    uu = np.ascontiguousarray(np.asarray(u, np.float32))
    df = np.ascontiguousarray(
        np.asarray(density_fine, np.float32).reshape(R_FULL, S))
    cf = np.ascontiguousarray(np.asarray(color_fine, np.float32))

    in_maps = []
    for c in range(NCORES):
        sl = slice(c * R_CORE, (c + 1) * R_CORE)
        in_maps.append({
            "density_coarse": dc[sl], "z_vals": zv[sl], "rays_d": rd[sl],
            "u": uu[sl], "density_fine": df[sl], "color_fine": cf[sl],
        })

    res = run_bass_kernel_spmd(nc, in_maps, core_ids=list(range(NCORES)))
    _CACHE["last_result"] = res
    outs = res.results
    rgb = np.concatenate([outs[c]["rgb"] for c in range(NCORES)], axis=0)
    acc = np.concatenate([outs[c]["acc"] for c in range(NCORES)], axis=0)
    dep = np.concatenate([outs[c]["depth"] for c in range(NCORES)], axis=0)
    zf = np.concatenate([outs[c]["z_fine"] for c in range(NCORES)], axis=0)
    return rgb, acc, dep, zf


def bench_device(inputs, iters=4):
    """Time device-side execution with device-resident inputs (excludes host
    transfers). Returns per-iteration seconds."""
    import time
    import jax
    import jax.numpy as jnp
    from jax.sharding import Mesh, PartitionSpec, NamedSharding
    from jax.experimental.shard_map import shard_map
    import concourse.mybir as mybir
    from concourse import bass2jax

    nc = _get_program()
    bass2jax.install_neuronx_cc_hook()

    partition_name = (nc.partition_id_tensor.name
                      if nc.partition_id_tensor else None)
    in_names, out_names, out_avals, zero_shapes = [], [], [], []
    for alloc in nc.m.functions[0].allocations:
        if not isinstance(alloc, mybir.MemoryLocationSet):
            continue
        name = alloc.memorylocations[0].name
        if alloc.kind == "ExternalInput":
            if name != partition_name:
                in_names.append(name)
        elif alloc.kind == "ExternalOutput":
            shape = tuple(alloc.tensor_shape)
            dtype = mybir.dt.np(alloc.dtype)
            out_names.append(name)
            out_avals.append(jax.core.ShapedArray(shape, dtype))
            zero_shapes.append((shape, dtype))
    n_params = len(in_names)
    all_names = in_names + out_names
    if partition_name is not None:
        all_names = all_names + [partition_name]

    def _body(*args):
        operands = list(args)
        if partition_name is not None:
            operands.append(bass2jax.partition_id_tensor())
        outs = bass2jax._bass_exec_p.bind(
            *operands, out_avals=tuple(out_avals), in_names=tuple(all_names),
            out_names=tuple(out_names), lowering_input_output_aliases=(),
            sim_require_finite=True, sim_require_nnan=True, nc=nc)
        return tuple(outs)

    devices = jax.devices()[:NCORES]
    mesh = Mesh(np.asarray(devices), ("core",))
    spec = PartitionSpec("core")
    sharded = jax.jit(
        shard_map(_body, mesh=mesh, in_specs=(spec,) * (n_params + len(out_names)),
                  out_specs=(spec,) * len(out_names), check_rep=False),
        donate_argnums=tuple(range(n_params, n_params + len(out_names))),
        keep_unused=True)

    dc = np.asarray(inputs["density_coarse"], np.float32).reshape(R_FULL, S)
    full = {"density_coarse": dc, "z_vals": inputs["z_vals"],
            "rays_d": inputs["rays_d"], "u": inputs["u"],
            "density_fine": np.asarray(inputs["density_fine"],
                                       np.float32).reshape(R_FULL, S),
            "color_fine": inputs["color_fine"]}
    sh = NamedSharding(mesh, spec)
    dev_in = [jax.device_put(np.asarray(full[n], np.float32), sh)
              for n in in_names]
    zeros_sets = []
    for _ in range(iters):
        zeros_sets.append([jax.device_put(
            np.zeros((NCORES * s[0], *s[1:]), d), sh) for s, d in zero_shapes])
    # warmup
    outs = sharded(*dev_in, *zeros_sets[0])
    jax.block_until_ready(outs)
    times = []
    for i in range(1, iters):
        t0 = time.perf_counter()
        outs = sharded(*dev_in, *zeros_sets[i])
        jax.block_until_ready(outs)
        times.append(time.perf_counter() - t0)
    return times
